# revision 28
# baseline (speedup 1.0000x reference)
"""BiMamba4TS Trainium2 Bass kernel.

Full-input contract: kernel(**inputs) takes the unsharded inputs from
setup_inputs() and returns the full [8, 4, 64, 62, 1] output.

Sharding: pure data parallel over the leading batch dim B=8 -> one batch
sample per NeuronCore.  Each core:
  - computes the SRA routing decision from its correlations slice on-device
  - folds the channel_independent/channel_mixing token select INTO the PE:
    tokT = xa^T @ ((1-f) I) + xb^T @ (f I) as two accumulating normal-mode
    matmuls against runtime-scaled fp32r identities (branch-free routing;
    normal-mode matmuls also keep the PE HAM clock warm -- transpose-mode
    does not count as PE-busy and runs throttled)
  - runs fwd+bwd mamba blocks: mm1 (x@W1+b1, silu) -> 3-tap conv across
    channels (6 accumulated 128x128x512 matmuls per output tile) + silu ->
    folded output projection (W2@Wr precomputed on host, so mm2 + the final
    einsum collapse into a single [F,1] dot)
  - the bwd direction's sequence flip is pure indexing (S is a batch dim for
    everything except the final sum), folded into the dot's read pattern.

All heavy matmuls use fp32r (1 cycle/row on TRN2 when the moving free dim
is >= 256, i.e. full 78.6 TF/s with fp32 storage).
"""

import contextlib

import numpy as np

import concourse.bass as bass
import concourse.tile as tile
from concourse import bacc, mybir
from concourse.masks import make_identity

# Problem shapes (hardcoded per contract)
B = 8
N1, S, L, P, F = 4, 64, 8192, 128, 256
LP = L // 128          # 64 patches per series
FH = 128               # half of F (PE partition limit)
CB = 512               # matmul moving-dim batch (columns)
NB = (S * LP) // CB    # 8 batches of 512 cols per n
OUTL = LP - 2          # 62 valid conv outputs per patch-block
NCORES = 8

F32 = mybir.dt.float32
F32R = mybir.dt.float32r
ALU = mybir.AluOpType
ACTF = mybir.ActivationFunctionType
AXX = mybir.AxisListType.X


def build_program():
    nc = bacc.Bacc("TRN2", target_bir_lowering=False, debug=False)

    x_d = nc.dram_tensor("x", [N1, S, L], F32R, kind="ExternalInput")
    corr_d = nc.dram_tensor("corr", [S, 1024], F32, kind="ExternalInput")
    w1_d = nc.dram_tensor("w1", [2, P, F], F32, kind="ExternalInput")
    cwt_d = nc.dram_tensor("cwt", [2, 3, 2, 2, FH, FH], F32, kind="ExternalInput")
    bp_d = nc.dram_tensor("biasp", [P, 13], F32, kind="ExternalInput")
    out_d = nc.dram_tensor("out", [N1, S, OUTL], F32, kind="ExternalOutput")

    x4 = x_d.ap().rearrange("n s (lp p) -> n s lp p", p=P)  # [4, 64, 64, 128]

    with tile.TileContext(nc) as tc:
        with contextlib.ExitStack() as ctx:
            _build_body(nc, tc, ctx, x4, corr_d, w1_d, cwt_d, bp_d, out_d)
    nc.compile()
    return nc


def _build_body(nc, tc, ctx, x4, corr_d, w1_d, cwt_d, bp_d, out_d):
    const = ctx.enter_context(tc.tile_pool(name="const", bufs=1))

    # ---- resident weights -------------------------------------------------
    # (the decide correlations ride the scalar queue FIRST -- the routing
    # flag gates the conditional token DMAs, so its latency is critical)
    corr_sb = const.tile([S, 1024], F32)
    nc.scalar.dma_start(out=corr_sb, in_=corr_d.ap())
    w1_sb = const.tile([P, 2, F], F32)
    nc.sync.dma_start(out=w1_sb, in_=w1_d.ap().rearrange("d p f -> p d f"))
    cwt_sb = const.tile([FH, 2, 3, 2, 2, FH], F32)
    nc.sync.dma_start(
        out=cwt_sb, in_=cwt_d.ap().rearrange("d k i o fi fo -> fi d k i o fo")
    )
    bp_sb = const.tile([P, 13], F32)
    nc.sync.dma_start(out=bp_sb, in_=bp_d.ap())
    ident = const.tile([P, P], F32)
    make_identity(nc, ident)
    # fp32r copies of all matmul weights (walrus requires fp32r matmul
    # operands to be produced -- i.e. rounded -- as fp32r)
    cwtr = const.tile([FH, 2, 3, 2, 2, FH], F32R)
    nc.vector.tensor_copy(out=cwtr, in_=cwt_sb)
    w2pr = const.tile([P, 4], F32R)
    nc.vector.tensor_copy(out=w2pr, in_=bp_sb[:, 8:12])
    zpad = const.tile([P, 2], F32)
    nc.vector.memset(zpad, 0.0)
    identr = const.tile([P, P], F32R)
    nc.vector.tensor_copy(out=identr, in_=ident)
    w1r = const.tile([P, 2, F], F32R)
    nc.vector.tensor_copy(out=w1r, in_=w1_sb)
    # integer routing flags (written by the decide stage) for the
    # conditional token DMAs: flag_i = f, flagn_i = 1-f
    flag_i = const.tile([1, 1], mybir.dt.int32)
    flagn_i = const.tile([1, 1], mybir.dt.int32)

    # ---- decide: SRA correlation routing ---------------------------------
    with (
        tc.tile_pool(name="dec", bufs=1) as dec,
        tc.tile_pool(name="decps", bufs=2, space="PSUM") as decps,
    ):
        c_t = corr_sb
        csum = dec.tile([S, 1], F32)
        nc.vector.reduce_sum(out=csum, in_=c_t, axis=AXX)
        cmean = dec.tile([S, 1], F32)
        nc.vector.tensor_scalar(
            out=cmean, in0=csum, scalar1=1.0 / 1024.0, scalar2=None, op0=ALU.mult
        )
        # centered (in place)
        nc.vector.tensor_scalar(
            out=c_t, in0=c_t, scalar1=cmean, scalar2=None, op0=ALU.subtract
        )
        sq = dec.tile([S, 1024], F32)
        nc.vector.tensor_tensor(out=sq, in0=c_t, in1=c_t, op=ALU.mult)
        ssq = dec.tile([S, 1], F32)
        nc.vector.reduce_sum(out=ssq, in_=sq, axis=AXX)
        stdv = dec.tile([S, 1], F32)
        # torch.std is unbiased: std = sqrt(ssq / (L-1))
        nc.scalar.activation(out=stdv, in_=ssq, func=ACTF.Sqrt, scale=1.0 / 1023.0)
        rstd = dec.tile([S, 1], F32)
        nc.vector.reciprocal(out=rstd, in_=stdv)
        nc.vector.tensor_scalar(
            out=c_t, in0=c_t, scalar1=rstd, scalar2=None, op0=ALU.mult
        )
        # gram matrix G = norm @ norm.T  (contract 1024 via 8 transposed blocks)
        normt = dec.tile([P, 512], F32)
        for k in range(8):
            tp = decps.tile([P, S], F32)
            nc.tensor.transpose(
                out=tp, in_=c_t[:, 128 * k : 128 * (k + 1)], identity=ident[0:S, 0:S]
            )
            nc.vector.tensor_copy(out=normt[:, S * k : S * (k + 1)], in_=tp)
        gps = decps.tile([S, S], F32)
        for k in range(8):
            nc.tensor.matmul(
                out=gps,
                lhsT=normt[:, S * k : S * (k + 1)],
                rhs=normt[:, S * k : S * (k + 1)],
                start=(k == 0),
                stop=(k == 7),
            )
        # counts: corr > 0.6  <=>  G > 0.6*1024 ;  corr > 0  <=>  G > 0
        c1 = dec.tile([S, S], F32)
        c0 = dec.tile([S, S], F32)
        nc.vector.tensor_scalar(
            out=c1, in0=gps, scalar1=0.6 * 1024.0, scalar2=None, op0=ALU.is_gt
        )
        nc.vector.tensor_scalar(
            out=c0, in0=gps, scalar1=0.0, scalar2=None, op0=ALU.is_gt
        )
        r1 = dec.tile([S, 2], F32)
        nc.vector.reduce_sum(out=r1[:, 0:1], in_=c1, axis=AXX)
        nc.vector.reduce_sum(out=r1[:, 1:2], in_=c0, axis=AXX)
        onescol = dec.tile([S, 1], F32)
        nc.vector.memset(onescol, 1.0)
        cntps = decps.tile([1, 2], F32)
        nc.tensor.matmul(out=cntps, lhsT=onescol, rhs=r1, start=True, stop=True)
        cnts = dec.tile([1, 2], F32)
        nc.vector.tensor_copy(out=cnts, in_=cntps)
        # ratio >= 0.4 with the diagonal (64 self-pairs) removed:
        #   (cnt_thr-64) >= 0.4*(cnt_pos-64)  <=>  cnt_thr - 0.4*cnt_pos >= 38.4
        t1 = dec.tile([1, 1], F32)
        nc.vector.tensor_scalar(
            out=t1, in0=cnts[:, 1:2], scalar1=-0.4, scalar2=None, op0=ALU.mult
        )
        t2 = dec.tile([1, 1], F32)
        nc.vector.tensor_tensor(out=t2, in0=cnts[:, 0:1], in1=t1, op=ALU.add)
        flag = dec.tile([1, 1], F32)
        nc.vector.tensor_scalar(
            out=flag, in0=t2, scalar1=38.3999, scalar2=None, op0=ALU.is_ge
        )
        # broadcast flag across partitions (K=1 matmul with a ones row)
        onesrow = dec.tile([1, P], F32)
        nc.vector.memset(onesrow, 1.0)
        fps = decps.tile([P, 1], F32)
        nc.tensor.matmul(out=fps, lhsT=onesrow, rhs=flag, start=True, stop=True)
        fvec = dec.tile([P, 1], F32)
        nc.vector.tensor_copy(out=fvec, in_=fps)
        onemf = dec.tile([P, 1], F32)
        nc.vector.tensor_scalar(
            out=onemf, in0=fvec, scalar1=-1.0, scalar2=1.0, op0=ALU.mult, op1=ALU.add
        )
        nc.vector.tensor_copy(out=flag_i, in_=flag)
        nc.vector.tensor_copy(out=flagn_i, in_=onemf[0:1, 0:1])

    # ---- persistent hT buffers (one n in flight) --------------------------
    hpool = ctx.enter_context(tc.tile_pool(name="ht", bufs=1))
    hbuf = {}
    for d in range(2):
        for i in range(2):
            t = hpool.tile([P, NB * CB + 2], F32R, name=f"ht_{d}_{i}")
            nc.vector.tensor_copy(out=t[:, NB * CB : NB * CB + 2], in_=zpad)
            hbuf[(d, i)] = t

    xa_p = ctx.enter_context(tc.tile_pool(name="xa", bufs=8))
    tk_ps = ctx.enter_context(tc.tile_pool(name="tkps", bufs=2, space="PSUM"))
    tok_p = ctx.enter_context(tc.tile_pool(name="tok", bufs=2))
    mm_ps = ctx.enter_context(tc.tile_pool(name="mmps", bufs=2, space="PSUM"))
    cv_ps = ctx.enter_context(tc.tile_pool(name="cvps", bufs=2, space="PSUM"))
    sf_p = ctx.enter_context(tc.tile_pool(name="sf", bufs=14))
    dt_ps = ctx.enter_context(tc.tile_pool(name="dtps", bufs=2, space="PSUM"))
    os_p = ctx.enter_context(tc.tile_pool(name="osb", bufs=4))

    # per-engine routing conds: registers loaded once from the decide flags
    from bass_rust import add_dep_helper

    sel = {}
    for eng in (nc.sync, nc.scalar):
        ra = ctx.enter_context(eng.register(name=f"ra_{eng.engine.name}"))
        rb = ctx.enter_context(eng.register(name=f"rb_{eng.engine.name}"))
        lda = eng.reg_load(ra, flagn_i[0:1, 0:1])
        ldb = eng.reg_load(rb, flag_i[0:1, 0:1])
        ca = eng.snap(ra, min_val=0, max_val=1)
        cb = eng.snap(rb, min_val=0, max_val=1)
        sel[eng] = (ca, cb, lda, ldb)

    def emit_chunk_pair(n, cp, tokt):
        """Routing-conditional load + transpose (normal-mode matmul against
        an fp32r identity) for chunks (2cp, 2cp+1): 4 s-tiles, 256 token
        columns.  Both layout DMAs are issued; the unselected one is skipped
        at runtime (semaphore still fires)."""
        tab = tk_ps.tile([P, 2, P], F32, tag="tab")  # one PSUM bank
        xa = xa_p.tile([P, 2, P], F32R)
        for j in range(2):
            c = 2 * cp + j
            eng_a = nc.sync if j == 0 else nc.scalar
            eng_b = nc.scalar if j == 0 else nc.sync
            ca, _, lda, _ = sel[eng_a]
            _, cb, _, ldb = sel[eng_b]
            ia = eng_a.dma_start(
                out=xa[:, j, :],
                in_=x4[n, 2 * c : 2 * c + 2].rearrange("s lp p -> (s lp) p"),
                cond=ca,
            )
            add_dep_helper(ia.ins, lda.ins, False, "cond reg ready")
            ib = eng_b.dma_start(
                out=xa[:, j, :],
                in_=x4[n, :, 2 * c : 2 * c + 2, :].rearrange("lp s p -> s lp p"),
                cond=cb,
            )
            add_dep_helper(ib.ins, ldb.ins, False, "cond reg ready")
            nc.tensor.matmul(
                out=tab[:, j, :], lhsT=xa[:, j, :], rhs=identr, start=True, stop=True
            )
        nc.vector.tensor_copy(
            out=tokt[:, 2 * P * cp : 2 * P * (cp + 1)].rearrange(
                "p (c h) -> p c h", h=P
            ),
            in_=tab,
        )

    def mm1_batch(n, tokt, bi):
        for d in range(2):
            for i in range(2):
                ps = mm_ps.tile([P, CB], F32)
                nc.tensor.matmul(
                    out=ps,
                    lhsT=w1r[:, d, i * FH : (i + 1) * FH],
                    rhs=tokt[:, CB * bi : CB * (bi + 1)],
                    start=True,
                    stop=True,
                )
                nc.scalar.activation(
                    out=hbuf[(d, i)][:, CB * bi : CB * (bi + 1)],
                    in_=ps,
                    func=ACTF.Silu,
                    bias=bp_sb[:, 2 * d + i : 2 * d + i + 1],
                    scale=1.0,
                )

    def conv_block(d, bi):
        """3-tap conv over patch positions + silu -> {fo_half: sf tile}."""
        sf = {}
        for o in range(2):
            ps = cv_ps.tile([P, CB], F32)
            first = True
            for i in range(2):
                for k in range(3):
                    nc.tensor.matmul(
                        out=ps,
                        lhsT=cwtr[:, d, k, i, o, :],
                        rhs=hbuf[(d, i)][:, CB * bi + k : CB * bi + k + CB],
                        start=first,
                        stop=(i == 1 and k == 2),
                    )
                    first = False
            t = sf_p.tile([P, CB], F32R)
            nc.scalar.activation(
                out=t,
                in_=ps,
                func=ACTF.Silu,
                bias=bp_sb[:, 4 + 2 * d + o : 5 + 2 * d + o],
                scale=1.0,
            )
            sf[o] = t
        return sf

    def flip_ap(t):
        """[128, 512] tile viewed with its 8 64-col blocks in reverse order."""
        a = t[:]
        return bass.AP(
            tensor=a.tensor,
            offset=a.offset + 7 * LP,
            ap=[a.ap[0], [-LP, 8], [1, LP]],
        )

    def dot_block(n, bi, sff, sfb):
        """Folded (W2 @ Wr) projection; bwd read s-flipped; +const; DMA out."""
        ps = dt_ps.tile([1, CB], F32)
        nc.tensor.matmul(
            out=ps, lhsT=w2pr[:, 0:1], rhs=sff[0], start=True, stop=False
        )
        nc.tensor.matmul(
            out=ps, lhsT=w2pr[:, 1:2], rhs=sff[1], start=False, stop=False
        )
        nc.tensor.matmul(
            out=ps, lhsT=w2pr[:, 2:3], rhs=flip_ap(sfb[0]), start=False, stop=False
        )
        nc.tensor.matmul(
            out=ps, lhsT=w2pr[:, 3:4], rhs=flip_ap(sfb[1]), start=False, stop=True
        )
        outs = os_p.tile([1, CB], F32)
        nc.scalar.activation(
            out=outs, in_=ps, func=ACTF.Identity, bias=bp_sb[0:1, 12:13], scale=1.0
        )
        ov = outs[:].rearrange("q (s l) -> q s l", l=LP)[:, :, 0:OUTL]
        nc.sync.dma_start(out=out_d.ap()[n, 8 * bi : 8 * bi + 8, :], in_=ov)

    def conv_phase(n, tokt_next):
        """conv+dot for n, with n+1 token chunk-pairs woven in (bursts of 4
        pairs, so tok matmuls don't fragment the conv accumulation stream)."""
        k = 0

        def weave4():
            nonlocal k
            if tokt_next is not None:
                for _ in range(4):
                    emit_chunk_pair(n + 1, k, tokt_next)
                    k += 1

        for half in (0, 1):
            lo = 4 * half
            sff = {}
            for b in range(lo, lo + 4):
                sff[b] = conv_block(0, b)
            weave4()
            for b in range(lo, lo + 4):
                bm = 7 - b
                sfb = conv_block(1, bm)
                dot_block(n, b, sff[b], sfb)
            weave4()

    # ---- main schedule ----------------------------------------------------
    tokts = [None] * (N1 + 1)
    tokts[0] = tok_p.tile([P, NB * CB], F32R, name="tokt0", tag="tokt")
    for cp in range(16):
        emit_chunk_pair(0, cp, tokts[0])
    for b in range(NB):
        mm1_batch(0, tokts[0], b)
    for n in range(N1):
        if n + 1 < N1:
            tokts[n + 1] = tok_p.tile(
                [P, NB * CB], F32R, name=f"tokt{n + 1}", tag="tokt"
            )
        if n > 0:
            for b in range(NB):
                mm1_batch(n, tokts[n], b)
        conv_phase(n, tokts[n + 1])


_PROGRAM = None


def _get_program():
    global _PROGRAM
    if _PROGRAM is None:
        _PROGRAM = build_program()
    return _PROGRAM


def _pack_weights(inputs):
    f32 = np.float32
    w1 = np.stack(
        [np.asarray(inputs["W1f"], f32), np.asarray(inputs["W1b"], f32)]
    )  # [2, P, F]
    cwt = np.empty((2, 3, 2, 2, FH, FH), f32)
    for d, key in enumerate(["Cwf", "Cwb"]):
        cw = np.asarray(inputs[key], f32)  # [F_out, F_in, 3]
        t = np.transpose(cw, (1, 0, 2))  # [fi, fo, k]
        for k in range(3):
            for i in range(2):
                for o in range(2):
                    cwt[d, k, i, o] = t[
                        i * FH : (i + 1) * FH, o * FH : (o + 1) * FH, k
                    ]
    wr = np.asarray(inputs["Wr"], f32)  # [F, 1]
    w2pf = np.asarray(inputs["W2f"], f32) @ wr  # [F, 1]
    w2pb = np.asarray(inputs["W2b"], f32) @ wr
    cconst = (
        np.asarray(inputs["b2f"], f32) @ wr
        + np.asarray(inputs["b2b"], f32) @ wr
        + np.asarray(inputs["br"], f32)
    ).item()
    bp = np.zeros((P, 13), f32)
    b1f = np.asarray(inputs["b1f"], f32)
    b1b = np.asarray(inputs["b1b"], f32)
    cbf = np.asarray(inputs["Cbf"], f32)
    cbb = np.asarray(inputs["Cbb"], f32)
    bp[:, 0] = b1f[:FH]
    bp[:, 1] = b1f[FH:]
    bp[:, 2] = b1b[:FH]
    bp[:, 3] = b1b[FH:]
    bp[:, 4] = cbf[:FH]
    bp[:, 5] = cbf[FH:]
    bp[:, 6] = cbb[:FH]
    bp[:, 7] = cbb[FH:]
    bp[:, 8] = w2pf[:FH, 0]
    bp[:, 9] = w2pf[FH:, 0]
    bp[:, 10] = w2pb[:FH, 0]
    bp[:, 11] = w2pb[FH:, 0]
    bp[:, 12] = cconst
    return w1, cwt, bp


def make_in_maps(inputs):
    x = np.ascontiguousarray(np.asarray(inputs["x"], np.float32))  # [8,4,64,8192]
    corr = np.ascontiguousarray(np.asarray(inputs["correlations"], np.float32))
    w1, cwt, bp = _pack_weights(inputs)
    return [
        {"x": x[b], "corr": corr[b], "w1": w1, "cwt": cwt, "biasp": bp}
        for b in range(NCORES)
    ]


def kernel(**inputs) -> np.ndarray:
    from concourse.bass_utils import run_bass_kernel_spmd

    nc = _get_program()
    in_maps = make_in_maps(inputs)
    res = run_bass_kernel_spmd(nc, in_maps, core_ids=list(range(NCORES)))
    out = np.stack([res.results[b]["out"] for b in range(NCORES)])
    return out[..., None].astype(np.float32)  # [8, 4, 64, 62, 1]
